# revision 6
# baseline (speedup 1.0000x reference)
"""BiGNN message-passing kernel for Trainium2 (8 NeuronCores, Bass/Tile).

Reference computation (N=100000 nodes, E=600000 edges, D=128):
    msgs = vals[:, None] * features[cols]            # gather + scale
    x    = segment_sum(msgs, rows)                   # scatter-add to rows
    out  = (features + x) @ W1 + b1 + (x * features) @ W2 + b2

Sharding: destination nodes (rows) are sharded across the 8 cores, 12500
each; `features` is replicated into every core's HBM, so the per-edge
source gather is core-local (no collectives).

Per core the edge set is reorganized on the host into 128-edge blocks,
each block targeting one 128-destination tile. The source-feature gather
runs on the GPSIMD `dma_gather` instruction, whose int16 indices limit a
single gather table to <32768 rows: `features` is therefore split into 4
column-chunks of N/4 rows (converted to fp16, 256B rows), and each
(dest-tile, col-chunk) edge group is padded to a multiple of 128 so that
gather-output blocks are dest-tile pure.

The segment-sum runs on TensorE as one matmul per block:

    xT[f, d] += G_blk[e, f].T @ S_blk[e, d]       (fp16 x fp16 -> f32 psum)
      G_blk = gathered source features (dma_gather output block)
      S_blk[e, d] = vals[e] * (dest_in_tile[e] == d)   (VectorE tensor_scalar
                    against a constant iota row, one op per block)

The dense epilogue stays in the transposed [feature, node] layout
(outT = W1.T @ (fT + xT) + W2.T @ (xT * fT) + (b1 + b2)), so no on-device
transposes are needed; the host transposes per-core outputs back.
"""

import numpy as np

P = 128
D = 128
N_NODES = 100000
N_EDGES = 600000
N_CORES = 8
NCHUNKS = 4  # feature-table column chunks (int16 index reach)
GROUP_TILES = 14  # dest tiles per gather/store group

_LAST_RESULTS = None  # BassKernelResults of the most recent run (for test.py)


def _prep(rows, cols, vals, n_nodes, n_cores):
    """Host-side edge reorganization into the shared block schedule.

    Returns (sched, per_core):
      sched: dict with tiles, cc, groups, per-tile block lists, NB, TOT
      per_core[c]: idx16 [128, TOT/16] int16, destP/valsP [128, NB] f32
    """
    npc = n_nodes // n_cores
    tiles = (npc + P - 1) // P
    cc = n_nodes // NCHUNKS
    assert n_nodes % NCHUNKS == 0

    rows = np.asarray(rows, dtype=np.int64)
    cols = np.asarray(cols, dtype=np.int64)
    vals = np.asarray(vals, dtype=np.float32)
    e = rows.shape[0]

    core = rows // npc
    local = rows - core * npc
    tile_idx = local // P
    dest_in_tile = (local - tile_idx * P).astype(np.float32)
    j_idx = cols // cc

    # group key in (core, tile, j) order
    key = (core * tiles + tile_idx) * NCHUNKS + j_idx
    order = np.argsort(key, kind="stable")
    key_s = key[order]
    cols_s = cols[order]
    dest_s = dest_in_tile[order]
    vals_s = vals[order]
    j_s = j_idx[order]

    cnt = np.bincount(key_s, minlength=n_cores * tiles * NCHUNKS).reshape(
        n_cores, tiles, NCHUNKS
    )
    starts_flat = np.concatenate([[0], np.cumsum(cnt.reshape(-1))[:-1]])
    rank = np.arange(e) - np.repeat(starts_flat, cnt.reshape(-1))

    # shared block counts per (tile, j): max over cores
    B = (cnt.max(axis=0) + P - 1) // P  # [tiles, NCHUNKS]
    # ensure every tile has at least one block (j=0)
    empty = B.sum(axis=1) == 0
    B[empty, 0] = 1

    # group structure: group g covers tiles [g0, g1); slot stream order is
    # g -> j -> t -> block. Compute per-(t, j) starting block id and slot.
    groups = []
    blk_of_tj = np.zeros((tiles, NCHUNKS), dtype=np.int64)  # global block id
    nb = 0
    for g0 in range(0, tiles, GROUP_TILES):
        g1 = min(g0 + GROUP_TILES, tiles)
        sections = []  # per j: (blk_start, nblk)
        for j in range(NCHUNKS):
            sec_start = nb
            for t in range(g0, g1):
                blk_of_tj[t, j] = nb
                nb += B[t, j]
            sections.append((sec_start, nb - sec_start))
        groups.append((g0, g1, sections))
    NB = nb
    TOT = NB * P

    # per-tile ordered block list: [(global_blk, j), ...] in j-then-block order
    tile_blocks = []
    for t in range(tiles):
        lst = []
        for j in range(NCHUNKS):
            for b in range(B[t, j]):
                lst.append((int(blk_of_tj[t, j] + b), j))
        tile_blocks.append(lst)

    # per-edge slot: blk_of_tj[t, j]*P + rank
    slot_s = blk_of_tj[tile_idx[order], j_s] * P + rank

    per_core = []
    core_s = core[order]
    for c in range(n_cores):
        m = core_s == c
        idx_flat = np.zeros(TOT, dtype=np.int16)
        dest_flat = np.zeros(TOT, dtype=np.float32)
        vals_flat = np.zeros(TOT, dtype=np.float32)
        s = slot_s[m]
        idx_flat[s] = (cols_s[m] - j_s[m] * cc).astype(np.int16)
        dest_flat[s] = dest_s[m]
        vals_flat[s] = vals_s[m]
        # idx stream wrapped in 16 partitions, replicated to 128
        idx16 = np.tile(np.ascontiguousarray(idx_flat.reshape(-1, 16).T), (8, 1))
        per_core.append(
            {
                "idx16": np.ascontiguousarray(idx16),
                "destP": np.ascontiguousarray(dest_flat.reshape(NB, P).T),
                "valsP": np.ascontiguousarray(vals_flat.reshape(NB, P).T),
            }
        )

    sched = {
        "tiles": tiles,
        "npc": npc,
        "cc": cc,
        "groups": groups,
        "tile_blocks": tile_blocks,
        "NB": NB,
        "TOT": TOT,
    }
    return sched, per_core


def _build_program(n_nodes, sched):
    import concourse.bacc as bacc
    import concourse.mybir as mybir
    import concourse.tile as tile

    f32 = mybir.dt.float32
    f16 = mybir.dt.float16
    i16 = mybir.dt.int16

    npc = sched["npc"]
    cc = sched["cc"]
    NB = sched["NB"]
    TOT = sched["TOT"]
    tile_blocks = sched["tile_blocks"]

    nc = bacc.Bacc()
    feat16 = [
        nc.dram_tensor(f"feat16_{j}", [cc, D], f16, kind="ExternalInput")
        for j in range(NCHUNKS)
    ]
    featT = nc.dram_tensor("featT", [D, npc], f32, kind="ExternalInput")
    w1 = nc.dram_tensor("W1", [D, D], f32, kind="ExternalInput")
    w2 = nc.dram_tensor("W2", [D, D], f32, kind="ExternalInput")
    bsum = nc.dram_tensor("bsum", [D, 1], f32, kind="ExternalInput")
    idx16 = nc.dram_tensor("idx16", [P, TOT // 16], i16, kind="ExternalInput")
    destP = nc.dram_tensor("destP", [P, NB], f32, kind="ExternalInput")
    valsP = nc.dram_tensor("valsP", [P, NB], f32, kind="ExternalInput")
    outT = nc.dram_tensor("outT", [D, npc], f32, kind="ExternalOutput")

    with tile.TileContext(nc) as tc:
        with (
            tc.tile_pool(name="const", bufs=1) as constp,
            tc.tile_pool(name="gpool", bufs=2) as gpool,
            tc.tile_pool(name="spool", bufs=4) as spool,
            tc.tile_pool(name="dense", bufs=3) as densep,
            tc.tile_pool(name="ostage", bufs=2) as ostagep,
            tc.tile_pool(name="psx", bufs=4, space="PSUM") as psx,
            tc.tile_pool(name="pso", bufs=2, space="PSUM") as pso,
        ):
            # --- constants ---
            iota_t = constp.tile([P, P], f32)
            nc.gpsimd.iota(
                iota_t[:],
                pattern=[[1, P]],
                base=0,
                channel_multiplier=0,
                allow_small_or_imprecise_dtypes=True,
            )
            w1_t = constp.tile([P, P], f32)
            nc.sync.dma_start(out=w1_t[:], in_=w1[:, :])
            w2_t = constp.tile([P, P], f32)
            nc.sync.dma_start(out=w2_t[:], in_=w2[:, :])
            bias_t = constp.tile([P, 1], f32)
            nc.sync.dma_start(out=bias_t[:], in_=bsum[:, :])
            idx16_t = constp.tile([P, TOT // 16], i16)
            nc.sync.dma_start(out=idx16_t[:], in_=idx16[:, :])
            destP_t = constp.tile([P, NB], f32)
            nc.sync.dma_start(out=destP_t[:], in_=destP[:, :])
            valsP_t = constp.tile([P, NB], f32)
            nc.sync.dma_start(out=valsP_t[:], in_=valsP[:, :])
            featT_t = constp.tile([P, npc], f32)
            nc.sync.dma_start(out=featT_t[:], in_=featT[:, :])

            for g0, g1, sections in sched["groups"]:
                gw = min(g1 * P, npc) - g0 * P  # group width in dest nodes

                # one dma_gather per feature-table chunk present in this group
                gtiles = {}
                for j in range(NCHUNKS):
                    sec_start, sec_nblk = sections[j]
                    if sec_nblk == 0:
                        continue
                    G = gpool.tile([P, sec_nblk, P], f16, tag=f"G{j}")
                    n_idx = sec_nblk * P
                    nc.gpsimd.dma_gather(
                        G[:],
                        feat16[j][:, :],
                        idx16_t[:, sec_start * 8 : sec_start * 8 + n_idx // 16],
                        n_idx,
                        n_idx,
                        D,
                        single_packet=False,
                    )
                    gtiles[j] = (G, sec_start)

                oT = ostagep.tile([P, gw], f32, tag="oT")

                for t in range(g0, g1):
                    w = min((t + 1) * P, npc) - t * P
                    blocks = tile_blocks[t]
                    xT = psx.tile([P, P], f32, tag="xT")
                    for bi, (blk, j) in enumerate(blocks):
                        S = spool.tile([P, P], f16, tag="S")
                        nc.vector.tensor_scalar(
                            out=S[:],
                            in0=iota_t[:],
                            scalar1=destP_t[:, blk : blk + 1],
                            scalar2=valsP_t[:, blk : blk + 1],
                            op0=mybir.AluOpType.is_equal,
                            op1=mybir.AluOpType.mult,
                        )
                        G, sec_start = gtiles[j]
                        nc.tensor.matmul(
                            out=xT[:],
                            lhsT=G[:, blk - sec_start, :],
                            rhs=S[:],
                            start=(bi == 0),
                            stop=(bi == len(blocks) - 1),
                        )

                    aT = densep.tile([P, P], f32, tag="aT")
                    mT = densep.tile([P, P], f32, tag="mT")
                    fslice = featT_t[:, t * P : t * P + w]
                    nc.vector.tensor_tensor(
                        out=aT[:, :w], in0=xT[:, :w], in1=fslice, op=mybir.AluOpType.add
                    )
                    nc.vector.tensor_tensor(
                        out=mT[:, :w], in0=xT[:, :w], in1=fslice, op=mybir.AluOpType.mult
                    )
                    out2 = pso.tile([P, P], f32, tag="out2")
                    nc.tensor.matmul(
                        out=out2[:, :w], lhsT=w1_t[:], rhs=aT[:, :w], start=True, stop=False
                    )
                    nc.tensor.matmul(
                        out=out2[:, :w], lhsT=w2_t[:], rhs=mT[:, :w], start=False, stop=True
                    )
                    nc.scalar.activation(
                        out=oT[:, (t - g0) * P : (t - g0) * P + w],
                        in_=out2[:, :w],
                        func=mybir.ActivationFunctionType.Identity,
                        bias=bias_t[:, :1],
                        scale=1.0,
                    )

                nc.sync.dma_start(out=outT[:, g0 * P : g0 * P + gw], in_=oT[:, :gw])
    nc.compile()
    return nc


def _run(rows, cols, vals, features, W1, b1, W2, b2, n_nodes, n_cores):
    global _LAST_RESULTS
    from concourse import bass_utils

    npc = n_nodes // n_cores
    features = np.ascontiguousarray(np.asarray(features, dtype=np.float32))
    W1 = np.ascontiguousarray(np.asarray(W1, dtype=np.float32))
    W2 = np.ascontiguousarray(np.asarray(W2, dtype=np.float32))
    bsum = np.ascontiguousarray(
        (np.asarray(b1, dtype=np.float32) + np.asarray(b2, dtype=np.float32)).reshape(
            D, 1
        )
    )

    sched, per_core = _prep(rows, cols, vals, n_nodes, n_cores)
    nc = _build_program(n_nodes, sched)

    cc = sched["cc"]
    feat16 = np.ascontiguousarray(features.astype(np.float16))
    feat16_chunks = [
        np.ascontiguousarray(feat16[j * cc : (j + 1) * cc, :]) for j in range(NCHUNKS)
    ]

    in_maps = []
    for c in range(n_cores):
        featT_c = np.ascontiguousarray(features[c * npc : (c + 1) * npc, :].T)
        im = {
            "featT": featT_c,
            "W1": W1,
            "W2": W2,
            "bsum": bsum,
            "idx16": per_core[c]["idx16"],
            "destP": per_core[c]["destP"],
            "valsP": per_core[c]["valsP"],
        }
        for j in range(NCHUNKS):
            im[f"feat16_{j}"] = feat16_chunks[j]
        in_maps.append(im)

    res = bass_utils.run_bass_kernel_spmd(nc, in_maps, core_ids=list(range(n_cores)))
    _LAST_RESULTS = res
    out = np.concatenate([r["outT"].T for r in res.results], axis=0)
    return np.ascontiguousarray(out)


def kernel(rows, cols, vals, features, W1, b1, W2, b2):
    return _run(rows, cols, vals, features, W1, b1, W2, b2, N_NODES, N_CORES)


# revision 15
# speedup vs baseline: 1.9645x; 1.9645x over previous
"""BiGNN message-passing kernel for Trainium2 (8 NeuronCores, Bass/Tile).

Reference computation (N=100000 nodes, E=600000 edges, D=128):
    msgs = vals[:, None] * features[cols]            # gather + scale
    x    = segment_sum(msgs, rows)                   # scatter-add to rows
    out  = (features + x) @ W1 + b1 + (x * features) @ W2 + b2

Sharding: destination nodes (rows) are sharded across the 8 cores, 12500
each; `features` is replicated into every core's HBM, so the per-edge
source gather is core-local (no collectives).

Per core the edge set is reorganized on the host into 128-edge blocks,
each block targeting one 128-destination tile. The source-feature gather
runs on the GPSIMD `dma_gather` instruction, whose int16 indices limit a
single gather table to <32768 rows: `features` is therefore split into 4
column-chunks of N/4 rows (converted to fp16, 256B rows), and each
(dest-tile, col-chunk) edge group is padded to a multiple of 128 so that
gather-output blocks are dest-tile pure.

The segment-sum runs on TensorE as one matmul per block:

    xT[f, d] += G_blk[e, f].T @ S_blk[e, d]       (fp16 x fp16 -> f32 psum)
      G_blk = gathered source features (dma_gather output block)
      S_blk[e, d] = vals[e] * (dest_in_tile[e] == d)   (VectorE tensor_scalar
                    against a constant iota row, one op per block)

The dense epilogue stays in the transposed [feature, node] layout
(outT = W1.T @ (fT + xT) + W2.T @ (xT * fT) + (b1 + b2)), so no on-device
transposes are needed; the host transposes per-core outputs back.
"""

import numpy as np

P = 128
D = 128
N_NODES = 100000
N_EDGES = 600000
N_CORES = 8
NCHUNKS = 4  # feature-table column chunks (int16 index reach)
GROUP_TILES = 14  # dest tiles per gather/store group

_LAST_RESULTS = None  # BassKernelResults of the most recent run (for test.py)


def _prep(rows, cols, vals, n_nodes, n_cores):
    """Host-side edge reorganization into the shared block schedule.

    Returns (sched, per_core):
      sched: dict with tiles, cc, groups, per-tile block lists, NB, TOT
      per_core[c]: idx16 [128, TOT/16] int16, destP/valsP [128, NB] f32
    """
    npc = n_nodes // n_cores
    tiles = (npc + P - 1) // P
    cc = n_nodes // NCHUNKS
    assert n_nodes % NCHUNKS == 0

    rows = np.asarray(rows, dtype=np.int64)
    cols = np.asarray(cols, dtype=np.int64)
    vals = np.asarray(vals, dtype=np.float32)
    e = rows.shape[0]

    core = rows // npc
    local = rows - core * npc
    tile_idx = local // P
    dest_in_tile = (local - tile_idx * P).astype(np.float32)
    j_idx = cols // cc

    # group key in (core, tile, j) order
    key = (core * tiles + tile_idx) * NCHUNKS + j_idx
    order = np.argsort(key, kind="stable")
    key_s = key[order]
    cols_s = cols[order]
    dest_s = dest_in_tile[order]
    vals_s = vals[order]
    j_s = j_idx[order]

    cnt = np.bincount(key_s, minlength=n_cores * tiles * NCHUNKS).reshape(
        n_cores, tiles, NCHUNKS
    )
    starts_flat = np.concatenate([[0], np.cumsum(cnt.reshape(-1))[:-1]])
    rank = np.arange(e) - np.repeat(starts_flat, cnt.reshape(-1))

    # shared block counts per (tile, j): max over cores
    B = (cnt.max(axis=0) + P - 1) // P  # [tiles, NCHUNKS]
    # ensure every tile has at least one block (j=0)
    empty = B.sum(axis=1) == 0
    B[empty, 0] = 1

    # group structure: group g covers tiles [g0, g1); slot stream order is
    # g -> j -> t -> block. Compute per-(t, j) starting block id and slot.
    groups = []
    blk_of_tj = np.zeros((tiles, NCHUNKS), dtype=np.int64)  # global block id
    nb = 0
    for g0 in range(0, tiles, GROUP_TILES):
        g1 = min(g0 + GROUP_TILES, tiles)
        sections = []  # per j: (blk_start, nblk)
        for j in range(NCHUNKS):
            sec_start = nb
            for t in range(g0, g1):
                blk_of_tj[t, j] = nb
                nb += B[t, j]
            sections.append((sec_start, nb - sec_start))
        groups.append((g0, g1, sections))
    NB = nb
    TOT = NB * P

    # per-tile ordered block list: [(global_blk, j), ...] in j-then-block order
    tile_blocks = []
    for t in range(tiles):
        lst = []
        for j in range(NCHUNKS):
            for b in range(B[t, j]):
                lst.append((int(blk_of_tj[t, j] + b), j))
        tile_blocks.append(lst)

    # per-edge slot: blk_of_tj[t, j]*P + rank
    slot_s = blk_of_tj[tile_idx[order], j_s] * P + rank

    per_core = []
    core_s = core[order]
    for c in range(n_cores):
        m = core_s == c
        idx_flat = np.zeros(TOT, dtype=np.int16)
        dest_flat = np.zeros(TOT, dtype=np.float32)
        vals_flat = np.zeros(TOT, dtype=np.float32)
        s = slot_s[m]
        idx_flat[s] = (cols_s[m] - j_s[m] * cc).astype(np.int16)
        dest_flat[s] = dest_s[m]
        vals_flat[s] = vals_s[m]
        # idx stream wrapped in 16 partitions, replicated to 128
        idx16 = np.tile(np.ascontiguousarray(idx_flat.reshape(-1, 16).T), (8, 1))
        per_core.append(
            {
                "idx16": np.ascontiguousarray(idx16),
                "destP": np.ascontiguousarray(dest_flat.reshape(NB, P).T),
                "valsP": np.ascontiguousarray(vals_flat.reshape(NB, P).T),
            }
        )

    sched = {
        "tiles": tiles,
        "npc": npc,
        "cc": cc,
        "groups": groups,
        "tile_blocks": tile_blocks,
        "NB": NB,
        "TOT": TOT,
    }
    return sched, per_core


def _build_program(n_nodes, sched):
    import concourse.bacc as bacc
    import concourse.mybir as mybir
    import concourse.tile as tile

    f32 = mybir.dt.float32
    f16 = mybir.dt.float16
    i16 = mybir.dt.int16

    npc = sched["npc"]
    cc = sched["cc"]
    NB = sched["NB"]
    TOT = sched["TOT"]
    tile_blocks = sched["tile_blocks"]

    nc = bacc.Bacc(num_swdge_queues=4)
    feat16 = [
        nc.dram_tensor(f"feat16_{j}", [cc, D], f16, kind="ExternalInput")
        for j in range(NCHUNKS)
    ]
    featT = nc.dram_tensor("featT", [D, npc], f32, kind="ExternalInput")
    w1 = nc.dram_tensor("W1", [D, D], f32, kind="ExternalInput")
    w2 = nc.dram_tensor("W2", [D, D], f32, kind="ExternalInput")
    bsum = nc.dram_tensor("bsum", [D, 1], f32, kind="ExternalInput")
    idx16 = nc.dram_tensor("idx16", [P, TOT // 16], i16, kind="ExternalInput")
    destP = nc.dram_tensor("destP", [P, NB], f32, kind="ExternalInput")
    valsP = nc.dram_tensor("valsP", [P, NB], f32, kind="ExternalInput")
    outT = nc.dram_tensor("outT", [D, npc], f32, kind="ExternalOutput")

    with tile.TileContext(nc) as tc:
        with (
            tc.tile_pool(name="const", bufs=1) as constp,
            tc.tile_pool(name="gpool", bufs=2) as gpool,
            tc.tile_pool(name="spool", bufs=4) as spool,
            tc.tile_pool(name="dense", bufs=3) as densep,
            tc.tile_pool(name="ostage", bufs=2) as ostagep,
            tc.tile_pool(name="psx", bufs=4, space="PSUM") as psx,
            tc.tile_pool(name="pso", bufs=2, space="PSUM") as pso,
        ):
            # --- constants ---
            iota_t = constp.tile([P, P], f16)
            nc.gpsimd.iota(
                iota_t[:],
                pattern=[[1, P]],
                base=0,
                channel_multiplier=0,
                allow_small_or_imprecise_dtypes=True,
            )
            w1_t = constp.tile([P, P], f32)
            nc.sync.dma_start(out=w1_t[:], in_=w1[:, :])
            w2_t = constp.tile([P, P], f32)
            nc.sync.dma_start(out=w2_t[:], in_=w2[:, :])
            bias_t = constp.tile([P, 1], f32)
            nc.sync.dma_start(out=bias_t[:], in_=bsum[:, :])
            idx16_t = constp.tile([P, TOT // 16], i16)
            nc.sync.dma_start(out=idx16_t[:], in_=idx16[:, :])
            destP_t = constp.tile([P, NB], f32)
            nc.sync.dma_start(out=destP_t[:], in_=destP[:, :])
            valsP_t = constp.tile([P, NB], f32)
            nc.sync.dma_start(out=valsP_t[:], in_=valsP[:, :])
            featT_t = constp.tile([P, npc], f32)
            nc.sync.dma_start(out=featT_t[:], in_=featT[:, :])

            for g0, g1, sections in sched["groups"]:
                gw = min(g1 * P, npc) - g0 * P  # group width in dest nodes

                # one dma_gather per feature-table chunk present in this group
                gtiles = {}
                for j in range(NCHUNKS):
                    sec_start, sec_nblk = sections[j]
                    if sec_nblk == 0:
                        continue
                    G = gpool.tile([P, sec_nblk, P], f16, tag=f"G{j}")
                    n_idx = sec_nblk * P
                    nc.gpsimd.dma_gather(
                        G[:],
                        feat16[j][:, :],
                        idx16_t[:, sec_start * 8 : sec_start * 8 + n_idx // 16],
                        n_idx,
                        n_idx,
                        D,
                        single_packet=False,
                        queue_num=j,
                    )
                    gtiles[j] = (G, sec_start)

                oT = ostagep.tile([P, gw], f32, tag="oT")

                for t in range(g0, g1):
                    w = min((t + 1) * P, npc) - t * P
                    blocks = tile_blocks[t]
                    xT = psx.tile([P, P], f32, tag="xT")
                    for bi, (blk, j) in enumerate(blocks):
                        S = spool.tile([P, P], f16, tag="S")
                        nc.vector.tensor_scalar(
                            out=S[:],
                            in0=iota_t[:],
                            scalar1=destP_t[:, blk : blk + 1],
                            scalar2=valsP_t[:, blk : blk + 1],
                            op0=mybir.AluOpType.is_equal,
                            op1=mybir.AluOpType.mult,
                        )
                        G, sec_start = gtiles[j]
                        nc.tensor.matmul(
                            out=xT[:],
                            lhsT=G[:, blk - sec_start, :],
                            rhs=S[:],
                            start=(bi == 0),
                            stop=(bi == len(blocks) - 1),
                        )

                    aT = densep.tile([P, P], f32, tag="aT")
                    mT = densep.tile([P, P], f32, tag="mT")
                    fslice = featT_t[:, t * P : t * P + w]
                    nc.vector.tensor_tensor(
                        out=aT[:, :w], in0=xT[:, :w], in1=fslice, op=mybir.AluOpType.add
                    )
                    nc.vector.tensor_tensor(
                        out=mT[:, :w], in0=xT[:, :w], in1=fslice, op=mybir.AluOpType.mult
                    )
                    out2 = pso.tile([P, P], f32, tag="out2")
                    nc.tensor.matmul(
                        out=out2[:, :w], lhsT=w1_t[:], rhs=aT[:, :w], start=True, stop=False
                    )
                    nc.tensor.matmul(
                        out=out2[:, :w], lhsT=w2_t[:], rhs=mT[:, :w], start=False, stop=True
                    )
                    nc.scalar.activation(
                        out=oT[:, (t - g0) * P : (t - g0) * P + w],
                        in_=out2[:, :w],
                        func=mybir.ActivationFunctionType.Identity,
                        bias=bias_t[:, :1],
                        scale=1.0,
                    )

                nc.sync.dma_start(out=outT[:, g0 * P : g0 * P + gw], in_=oT[:, :gw])
    nc.compile()
    return nc


def _run(rows, cols, vals, features, W1, b1, W2, b2, n_nodes, n_cores):
    global _LAST_RESULTS
    from concourse import bass_utils

    npc = n_nodes // n_cores
    features = np.ascontiguousarray(np.asarray(features, dtype=np.float32))
    W1 = np.ascontiguousarray(np.asarray(W1, dtype=np.float32))
    W2 = np.ascontiguousarray(np.asarray(W2, dtype=np.float32))
    bsum = np.ascontiguousarray(
        (np.asarray(b1, dtype=np.float32) + np.asarray(b2, dtype=np.float32)).reshape(
            D, 1
        )
    )

    sched, per_core = _prep(rows, cols, vals, n_nodes, n_cores)
    nc = _build_program(n_nodes, sched)

    cc = sched["cc"]
    feat16 = np.ascontiguousarray(features.astype(np.float16))
    feat16_chunks = [
        np.ascontiguousarray(feat16[j * cc : (j + 1) * cc, :]) for j in range(NCHUNKS)
    ]

    in_maps = []
    for c in range(n_cores):
        featT_c = np.ascontiguousarray(features[c * npc : (c + 1) * npc, :].T)
        im = {
            "featT": featT_c,
            "W1": W1,
            "W2": W2,
            "bsum": bsum,
            "idx16": per_core[c]["idx16"],
            "destP": per_core[c]["destP"],
            "valsP": per_core[c]["valsP"],
        }
        for j in range(NCHUNKS):
            im[f"feat16_{j}"] = feat16_chunks[j]
        in_maps.append(im)

    res = bass_utils.run_bass_kernel_spmd(nc, in_maps, core_ids=list(range(n_cores)))
    _LAST_RESULTS = res
    out = np.concatenate([r["outT"].T for r in res.results], axis=0)
    return np.ascontiguousarray(out)


def kernel(rows, cols, vals, features, W1, b1, W2, b2):
    return _run(rows, cols, vals, features, W1, b1, W2, b2, N_NODES, N_CORES)


# revision 16
# speedup vs baseline: 2.8358x; 1.4435x over previous
"""BiGNN message-passing kernel for Trainium2 (8 NeuronCores, Bass/Tile).

Reference computation (N=100000 nodes, E=600000 edges, D=128):
    msgs = vals[:, None] * features[cols]            # gather + scale
    x    = segment_sum(msgs, rows)                   # scatter-add to rows
    out  = (features + x) @ W1 + b1 + (x * features) @ W2 + b2

Sharding: destination nodes (rows) are sharded across the 8 cores, 12500
each; `features` is replicated into every core's HBM, so the per-edge
source gather is core-local (no collectives).

Per core the edge set is reorganized on the host into 128-edge blocks,
each block targeting one 128-destination tile. The source-feature gather
runs on the GPSIMD `dma_gather` instruction (4 parallel SWDGE queues),
whose int16 indices limit a single gather table to <32768 rows:
`features` is split into 4 column-chunks of N/4 rows (fp16, 256B rows),
and each (dest-tile, col-chunk) edge group is padded to a multiple of
128 so gather-output blocks stay dest-tile pure.

The segment-sum runs on TensorE as one matmul per block:

    xT[f, d] += G_blk[e, f].T @ S_blk[e, d]       (fp16 x fp16 -> f32 psum)
      G_blk = gathered source features (dma_gather output block)
      S_blk[e, d] = vals[e] * (dest_in_tile[e] == d)

S blocks are built on the host (one fp16 one-hot row per edge) and
streamed in by group — cheaper than building them on VectorE, which
was the measured bottleneck.

The dense epilogue stays in the transposed [feature, node] layout
(outT = W1.T @ (fT + xT) + W2.T @ (xT * fT) + (b1 + b2)), so no
on-device transposes are needed; the host transposes per-core outputs
back. W1/W2 and the (f+x)/(x*f) operands run in fp16 on the PE.
"""

import numpy as np

P = 128
D = 128
N_NODES = 100000
N_EDGES = 600000
N_CORES = 8
NCHUNKS = 4  # feature-table column chunks (int16 index reach)
GROUP_TILES = 8  # dest tiles per gather/store group

_LAST_RESULTS = None  # BassKernelResults of the most recent run (for test.py)


def _prep(rows, cols, vals, n_nodes, n_cores):
    """Host-side edge reorganization into the shared block schedule.

    Returns (sched, per_core):
      sched: tiles/npc/cc/groups/tile_blocks/NB/TOT
      per_core[c]: idx16 [128, TOT/16] int16, S16 [128, NB*128] fp16
    """
    npc = n_nodes // n_cores
    tiles = (npc + P - 1) // P
    cc = n_nodes // NCHUNKS
    assert n_nodes % NCHUNKS == 0

    rows = np.asarray(rows, dtype=np.int64)
    cols = np.asarray(cols, dtype=np.int64)
    vals = np.asarray(vals, dtype=np.float32)
    e = rows.shape[0]

    core = rows // npc
    local = rows - core * npc
    tile_idx = local // P
    dest_in_tile = (local - tile_idx * P).astype(np.int64)
    j_idx = cols // cc

    key = (core * tiles + tile_idx) * NCHUNKS + j_idx
    order = np.argsort(key, kind="stable")
    cols_s = cols[order]
    dest_s = dest_in_tile[order]
    vals_s = vals[order]
    j_s = j_idx[order]

    cnt = np.bincount(key[order], minlength=n_cores * tiles * NCHUNKS).reshape(
        n_cores, tiles, NCHUNKS
    )
    starts_flat = np.concatenate([[0], np.cumsum(cnt.reshape(-1))[:-1]])
    rank = np.arange(e) - np.repeat(starts_flat, cnt.reshape(-1))

    B = (cnt.max(axis=0) + P - 1) // P  # blocks per (tile, j), shared
    empty = B.sum(axis=1) == 0
    B[empty, 0] = 1

    groups = []
    blk_of_tj = np.zeros((tiles, NCHUNKS), dtype=np.int64)
    nb = 0
    for g0 in range(0, tiles, GROUP_TILES):
        g1 = min(g0 + GROUP_TILES, tiles)
        sections = []
        for j in range(NCHUNKS):
            sec_start = nb
            for t in range(g0, g1):
                blk_of_tj[t, j] = nb
                nb += B[t, j]
            sections.append((sec_start, nb - sec_start))
        groups.append((g0, g1, sections))
    NB = nb
    TOT = NB * P

    tile_blocks = []
    for t in range(tiles):
        lst = []
        for j in range(NCHUNKS):
            for b in range(B[t, j]):
                lst.append((int(blk_of_tj[t, j] + b), j))
        tile_blocks.append(lst)

    slot_s = blk_of_tj[tile_idx[order], j_s] * P + rank

    per_core = []
    core_s = core[order]
    for c in range(n_cores):
        m = core_s == c
        s = slot_s[m]
        idx_flat = np.zeros(TOT, dtype=np.int16)
        idx_flat[s] = (cols_s[m] - j_s[m] * cc).astype(np.int16)
        idx16 = np.tile(np.ascontiguousarray(idx_flat.reshape(-1, 16).T), (8, 1))
        # S blocks: S[b, e, d] = val * (dest == d); packed to [128e, NB*128d]
        S_all = np.zeros((NB, P, P), dtype=np.float16)
        S_all[s // P, s % P, dest_s[m]] = vals_s[m].astype(np.float16)
        S16 = np.ascontiguousarray(S_all.transpose(1, 0, 2).reshape(P, NB * P))
        per_core.append({"idx16": np.ascontiguousarray(idx16), "S16": S16})

    sched = {
        "tiles": tiles,
        "npc": npc,
        "cc": cc,
        "groups": groups,
        "tile_blocks": tile_blocks,
        "NB": NB,
        "TOT": TOT,
    }
    return sched, per_core


def _build_program(n_nodes, sched):
    import concourse.bacc as bacc
    import concourse.mybir as mybir
    import concourse.tile as tile

    f32 = mybir.dt.float32
    f16 = mybir.dt.float16
    i16 = mybir.dt.int16

    npc = sched["npc"]
    cc = sched["cc"]
    NB = sched["NB"]
    TOT = sched["TOT"]
    tile_blocks = sched["tile_blocks"]

    nc = bacc.Bacc(num_swdge_queues=4)
    feat16 = [
        nc.dram_tensor(f"feat16_{j}", [cc, D], f16, kind="ExternalInput")
        for j in range(NCHUNKS)
    ]
    featT = nc.dram_tensor("featT", [D, npc], f32, kind="ExternalInput")
    w1 = nc.dram_tensor("W1", [D, D], f16, kind="ExternalInput")
    w2 = nc.dram_tensor("W2", [D, D], f16, kind="ExternalInput")
    bsum = nc.dram_tensor("bsum", [D, 1], f32, kind="ExternalInput")
    idx16 = nc.dram_tensor("idx16", [P, TOT // 16], i16, kind="ExternalInput")
    s16 = nc.dram_tensor("S16", [P, NB * P], f16, kind="ExternalInput")
    outT = nc.dram_tensor("outT", [D, npc], f32, kind="ExternalOutput")

    with tile.TileContext(nc) as tc:
        with (
            tc.tile_pool(name="const", bufs=1) as constp,
            tc.tile_pool(name="gpool", bufs=2) as gpool,
            tc.tile_pool(name="spool", bufs=2) as spool,
            tc.tile_pool(name="dense", bufs=3) as densep,
            tc.tile_pool(name="ostage", bufs=2) as ostagep,
            tc.tile_pool(name="psx", bufs=4, space="PSUM") as psx,
            tc.tile_pool(name="pso", bufs=2, space="PSUM") as pso,
        ):
            # --- constants ---
            w1_t = constp.tile([P, P], f16)
            nc.sync.dma_start(out=w1_t[:], in_=w1[:, :])
            w2_t = constp.tile([P, P], f16)
            nc.sync.dma_start(out=w2_t[:], in_=w2[:, :])
            bias_t = constp.tile([P, 1], f32)
            nc.sync.dma_start(out=bias_t[:], in_=bsum[:, :])
            idx16_t = constp.tile([P, TOT // 16], i16)
            nc.sync.dma_start(out=idx16_t[:], in_=idx16[:, :])
            featT_t = constp.tile([P, npc], f32)
            nc.sync.dma_start(out=featT_t[:], in_=featT[:, :])

            for g0, g1, sections in sched["groups"]:
                gw = min(g1 * P, npc) - g0 * P
                ch0 = sections[0][0]
                ch1 = sections[-1][0] + sections[-1][1]
                nch = ch1 - ch0

                # S blocks for this group, one sequential DMA
                S = spool.tile([P, nch * P], f16, tag="S")
                nc.sync.dma_start(out=S[:], in_=s16[:, ch0 * P : ch1 * P])

                # one dma_gather per feature-table chunk, parallel SWDGE queues
                gtiles = {}
                for j in range(NCHUNKS):
                    sec_start, sec_nblk = sections[j]
                    if sec_nblk == 0:
                        continue
                    G = gpool.tile([P, sec_nblk, P], f16, tag=f"G{j}")
                    n_idx = sec_nblk * P
                    nc.gpsimd.dma_gather(
                        G[:],
                        feat16[j][:, :],
                        idx16_t[:, sec_start * 8 : sec_start * 8 + n_idx // 16],
                        n_idx,
                        n_idx,
                        D,
                        single_packet=False,
                        queue_num=j,
                    )
                    gtiles[j] = (G, sec_start)

                oT = ostagep.tile([P, gw], f32, tag="oT")

                for t in range(g0, g1):
                    w = min((t + 1) * P, npc) - t * P
                    blocks = tile_blocks[t]
                    xT = psx.tile([P, P], f32, tag="xT")
                    for bi, (blk, j) in enumerate(blocks):
                        G, sec_start = gtiles[j]
                        nc.tensor.matmul(
                            out=xT[:],
                            lhsT=G[:, blk - sec_start, :],
                            rhs=S[:, (blk - ch0) * P : (blk - ch0 + 1) * P],
                            start=(bi == 0),
                            stop=(bi == len(blocks) - 1),
                        )

                    aT = densep.tile([P, P], f16, tag="aT")
                    mT = densep.tile([P, P], f16, tag="mT")
                    fslice = featT_t[:, t * P : t * P + w]
                    nc.vector.tensor_tensor(
                        out=aT[:, :w], in0=xT[:, :w], in1=fslice, op=mybir.AluOpType.add
                    )
                    nc.vector.tensor_tensor(
                        out=mT[:, :w], in0=xT[:, :w], in1=fslice, op=mybir.AluOpType.mult
                    )
                    out2 = pso.tile([P, P], f32, tag="out2")
                    nc.tensor.matmul(
                        out=out2[:, :w], lhsT=w1_t[:], rhs=aT[:, :w], start=True, stop=False
                    )
                    nc.tensor.matmul(
                        out=out2[:, :w], lhsT=w2_t[:], rhs=mT[:, :w], start=False, stop=True
                    )
                    nc.scalar.activation(
                        out=oT[:, (t - g0) * P : (t - g0) * P + w],
                        in_=out2[:, :w],
                        func=mybir.ActivationFunctionType.Identity,
                        bias=bias_t[:, :1],
                        scale=1.0,
                    )

                nc.sync.dma_start(out=outT[:, g0 * P : g0 * P + gw], in_=oT[:, :gw])
    nc.compile()
    return nc


def _run(rows, cols, vals, features, W1, b1, W2, b2, n_nodes, n_cores):
    global _LAST_RESULTS
    from concourse import bass_utils

    npc = n_nodes // n_cores
    features = np.ascontiguousarray(np.asarray(features, dtype=np.float32))
    W1_16 = np.ascontiguousarray(np.asarray(W1, dtype=np.float32).astype(np.float16))
    W2_16 = np.ascontiguousarray(np.asarray(W2, dtype=np.float32).astype(np.float16))
    bsum = np.ascontiguousarray(
        (np.asarray(b1, dtype=np.float32) + np.asarray(b2, dtype=np.float32)).reshape(
            D, 1
        )
    )

    sched, per_core = _prep(rows, cols, vals, n_nodes, n_cores)
    nc = _build_program(n_nodes, sched)

    cc = sched["cc"]
    feat16 = np.ascontiguousarray(features.astype(np.float16))
    feat16_chunks = [
        np.ascontiguousarray(feat16[j * cc : (j + 1) * cc, :]) for j in range(NCHUNKS)
    ]

    in_maps = []
    for c in range(n_cores):
        featT_c = np.ascontiguousarray(features[c * npc : (c + 1) * npc, :].T)
        im = {
            "featT": featT_c,
            "W1": W1_16,
            "W2": W2_16,
            "bsum": bsum,
            "idx16": per_core[c]["idx16"],
            "S16": per_core[c]["S16"],
        }
        for j in range(NCHUNKS):
            im[f"feat16_{j}"] = feat16_chunks[j]
        in_maps.append(im)

    res = bass_utils.run_bass_kernel_spmd(nc, in_maps, core_ids=list(range(n_cores)))
    _LAST_RESULTS = res
    out = np.concatenate([r["outT"].T for r in res.results], axis=0)
    return np.ascontiguousarray(out)


def kernel(rows, cols, vals, features, W1, b1, W2, b2):
    return _run(rows, cols, vals, features, W1, b1, W2, b2, N_NODES, N_CORES)
